# revision 25
# baseline (speedup 1.0000x reference)
"""Trainium2 Bass kernel for CellSizePredictor (v7: deferred atomic reduction +
granular startup).

reference:
    average = x[:, :n]; numbers = x[:, n:]
    o = numbers * average**alpha
    out = o @ A + einsum('bi,ij,bj->b', o, B, o) + C

Design (data-parallel over 8 cores, batch shard 8192 rows each):
  * Host pre-transposes each x shard to xT [2048, 8192] fp16; device
    streams feature-major tiles with contiguous DMAs.
  * Host folds the quadratic form into U = triu(B+B^T,1)+diag(B) so
    quad_b = sum_j o_bj (o@U)_bj; PE runs 36 of 64 [128x128x512] fp16
    matmul tiles per batch sub-chunk.
  * Epilogue off the critical engines: ACT does z16_j = Identity(p_z_j
    + A_j) from PSUM (per-partition bias); DVE does fp16 SBUF 2x ops
    only (o-mul, psT = z16*oT, pair/quad adds); ACT adds C.
  * Partition reduction: psT pairs -> quads -> one oct tile on DVE,
    then a SINGLE complete ones-row matmul group per sub-chunk.  The
    reduction matmuls + C-add + output DMA of super-chunk s are
    emitted after super-chunk s+1's j=2 group, so the PE queue never
    waits on the ACT->DVE tail chain (v5 traced a ~2us PE stall per
    boundary; split start/stop groups across the boundary wedge the
    device, so the group stays atomic).
  * Startup: super-chunk 0 uses granular per-chunk DMAs/o-muls so the
    first matmuls unlock ASAP; U arrives as chunks {0,1}, {2,3},
    {4..7}; later super-chunks use batched [128,2,SUP] pair-DMAs and
    [128, 2*SUP] o-muls (fewer Sync-queue issues).  40 dummy matmuls
    warm the PE HAM clock gate; a dummy activation pre-loads the ACT
    Identity table during the DMA ramp.
"""
import sys

for _p in ("/opt/trn_rl_repo",):
    if _p not in sys.path:
        sys.path.append(_p)

import numpy as np
from contextlib import ExitStack

import concourse.bass as bass
import concourse.tile as tile
from concourse import bacc, mybir
from concourse.bass_utils import run_bass_kernel_spmd

dt = mybir.dt
F32 = dt.float32
F16 = dt.float16

N_CORES = 8
BATCH = 65536
N = 1024
SHARD = BATCH // N_CORES          # 8192
N_IC = N // 128                   # 8 contraction chunks of 128
SUP = 1024                        # batch rows per load super-chunk
BCH = 512                         # batch rows per compute chunk (matmul N)
N_SUP = SHARD // SUP              # 8
SUB = SUP // BCH                  # 2
N_WARM = 80                       # PE warm-up dummy matmuls
IDENT = mybir.ActivationFunctionType.Identity


def _build(n_sup: int):
    nc = bacc.Bacc("TRN2", target_bir_lowering=False, debug=False)

    rows = n_sup * SUP
    x_d = nc.dram_tensor("xt", [2 * N, rows], F16, kind="ExternalInput").ap()
    u_d = nc.dram_tensor("u", [N, N], F16, kind="ExternalInput").ap()
    a_d = nc.dram_tensor("a2", [128, N_IC], F32, kind="ExternalInput").ap()
    c_d = nc.dram_tensor("c1", [1, 1], F32, kind="ExternalInput").ap()
    out_d = nc.dram_tensor("out", [rows], F32, kind="ExternalOutput").ap()
    out_2d = out_d.rearrange("(a b) -> a b", a=1)
    x_3d = x_d.rearrange("(c p) b -> p c b", p=128)   # [128, 16, rows]
    u_3d = u_d.rearrange("(c p) n -> p c n", p=128)   # [128, 8, N]

    with tile.TileContext(nc) as tc, ExitStack() as ctx:
        consts = ctx.enter_context(tc.tile_pool(name="consts", bufs=1))
        xin = ctx.enter_context(tc.tile_pool(name="xin", bufs=2))
        opool = ctx.enter_context(tc.tile_pool(name="opool", bufs=2))
        zpool = ctx.enter_context(tc.tile_pool(name="zpool", bufs=2))
        ppool = ctx.enter_context(tc.tile_pool(name="ppool", bufs=2))
        qpool = ctx.enter_context(tc.tile_pool(name="qpool", bufs=2))
        tpool = ctx.enter_context(tc.tile_pool(name="tpool", bufs=1))
        ps_z = ctx.enter_context(tc.tile_pool(name="ps_z", bufs=3, space="PSUM"))
        ps_r = ctx.enter_context(tc.tile_pool(name="ps_r", bufs=1, space="PSUM"))

        # ---- PE warm-up (HAM clock gate) + ACT table pre-load ----
        warm16 = consts.tile([128, 64], F16)
        nc.vector.memset(warm16[:], 0.0)
        p_warm = ps_z.tile([128, SUP], F32, tag="pz")
        for _ in range(N_WARM):
            nc.tensor.matmul(p_warm[0:64, 0:64], warm16[:], warm16[:],
                             start=True, stop=True)
        # ---- constants ----
        u_all = consts.tile([128, N_IC * N], F16)
        u_sb = [u_all[:, i * N : (i + 1) * N] for i in range(N_IC)]
        a_sb = consts.tile([128, N_IC], F32)
        c_sb = consts.tile([1, 1], F32)
        ones_f = consts.tile([128, 1], F32)
        nc.vector.memset(ones_f[:], 1.0)
        ones_h = consts.tile([128, 1], F16)
        nc.vector.tensor_copy(ones_h[:], ones_f[:])
        out_sb = consts.tile([1, rows], F32)
        # pre-load the ACT Identity table during the DMA ramp
        actwarm = consts.tile([1, 1], F32)
        nc.scalar.activation(actwarm[:], ones_f[0:1, 0:1], IDENT, bias=0.0)

        prev = None  # deferred tail of the previous super-chunk

        def flush_prev():
            # deferred tail of the previous super-chunk: one COMPLETE
            # single-matmul reduction group per sub (no split groups),
            # then +C on ACT and the output DMA
            nonlocal prev
            if prev is None:
                return
            oct_t, pr0 = prev
            pres = [
                ps_r.tile([1, BCH], F32, tag=f"pres{sub}", name=f"pres{sub}")
                for sub in range(SUB)
            ]
            for sub in range(SUB):
                zsl = slice(sub * BCH, (sub + 1) * BCH)
                nc.tensor.matmul(
                    pres[sub][:], ones_h[:], oct_t[:, zsl],
                    start=True, stop=True,
                )
            for sub in range(SUB):
                b0 = pr0 + sub * BCH
                nc.scalar.activation(
                    out_sb[0:1, b0 : b0 + BCH], pres[sub][:], IDENT,
                    bias=c_sb[0:1, 0:1],
                )
            nc.sync.dma_start(
                out_2d[0:1, pr0 : pr0 + SUP], out_sb[0:1, pr0 : pr0 + SUP]
            )
            prev = None

        for sc in range(n_sup):
            r0 = sc * SUP
            oT = []
            if sc == 0:
                # granular loads: chunk-at-a-time so j groups unlock ASAP
                xs = []
                os_ = []
                for i in range(N_IC):
                    if i % 2 == 0:
                        at = xin.tile([128, 2 * SUP], F16, tag=f"avg{i // 2}")
                        nt = xin.tile([128, 2 * SUP], F16, tag=f"num{i // 2}")
                        xs.append((at, nt))
                        ok = opool.tile([128, 2 * SUP], F16, tag=f"o{i // 2}")
                        os_.append(ok)
                    at, nt = xs[-1]
                    ok = os_[-1]
                    h = slice((i % 2) * SUP, (i % 2 + 1) * SUP)
                    # U chunk i, TRIANGULAR: only columns >= i*128 are
                    # ever read by the matmuls (36/64 tiles), the rest
                    # stays uninitialized — 1.125 MB instead of 2 MB
                    # on the startup-critical DMA path
                    nc.sync.dma_start(
                        u_all[:, i * N + i * 128 : (i + 1) * N],
                        u_d[i * 128 : (i + 1) * 128, i * 128 :],
                    )
                    nc.sync.dma_start(
                        at[:, h], x_d[i * 128 : (i + 1) * 128, r0 : r0 + SUP]
                    )
                    nc.sync.dma_start(
                        nt[:, h],
                        x_d[N + i * 128 : N + (i + 1) * 128, r0 : r0 + SUP],
                    )
                    if i == 0:
                        nc.sync.dma_start(a_sb[:], a_d)
                        nc.sync.dma_start(c_sb[:], c_d)
                    # per-chunk o-mul so oT_i is ready incrementally
                    nc.vector.tensor_mul(ok[:, h], at[:, h], nt[:, h])
                    oT.append(ok[:, h])
                oT2 = [t[:] for t in os_]
            else:
                # batched pair loads + double o-muls
                avg2, num2 = [], []
                for k in range(N_IC // 2):
                    at = xin.tile([128, 2 * SUP], F16, tag=f"avg{k}")
                    nc.sync.dma_start(
                        at[:].rearrange("p (c b) -> p c b", c=2),
                        x_3d[:, 2 * k : 2 * k + 2, r0 : r0 + SUP],
                    )
                    avg2.append(at)
                    nt = xin.tile([128, 2 * SUP], F16, tag=f"num{k}")
                    nc.sync.dma_start(
                        nt[:].rearrange("p (c b) -> p c b", c=2),
                        x_3d[:, N_IC + 2 * k : N_IC + 2 * k + 2,
                             r0 : r0 + SUP],
                    )
                    num2.append(nt)
                oT2 = []
                for k in range(N_IC // 2):
                    ok = opool.tile([128, 2 * SUP], F16, tag=f"o{k}")
                    nc.vector.tensor_mul(ok[:], avg2[k][:], num2[k][:])
                    oT.append(ok[:, 0:SUP])
                    oT.append(ok[:, SUP : 2 * SUP])
                    oT2.append(ok[:])

            is_last = sc == n_sup - 1
            psTs = []
            psTs_s = {}
            pairs = []
            quad0 = None
            for j in range(N_IC):
                jsl = slice(j * 128, (j + 1) * 128)
                p_z = ps_z.tile([128, SUP], F32, tag="pz")
                for sub in range(SUB):
                    zsl = slice(sub * BCH, (sub + 1) * BCH)
                    for i in range(j + 1):
                        nc.tensor.matmul(
                            p_z[:, zsl],
                            u_sb[i][:, jsl],
                            oT[i][:, zsl],
                            start=(i == 0),
                            stop=(i == j),
                        )
                if j == 5:
                    # previous super-chunk's tail rides behind our j<=4
                    # groups (Tile often runs j<=2 early, overlapped
                    # with the previous tail): flush late enough that
                    # the ACT->DVE chain is done when PE reaches it
                    flush_prev()
                if is_last and j >= 6:
                    # last super-chunk: per-sub epilogue chain so the
                    # final reduction waits on a half-width tail
                    for sub in range(SUB):
                        zsl = slice(sub * BCH, (sub + 1) * BCH)
                        z16s = tpool.tile([128, BCH], F16,
                                          tag=f"z16s{j}_{sub}",
                                          name=f"z16s{j}_{sub}")
                        nc.scalar.activation(z16s[:], p_z[:, zsl], IDENT,
                                             bias=a_sb[:, j : j + 1])
                        psT_s = tpool.tile([128, BCH], F16,
                                           tag=f"psTs{j}_{sub}",
                                           name=f"psTs{j}_{sub}")
                        nc.vector.tensor_mul(psT_s[:], z16s[:],
                                             oT[j][:, zsl])
                        psTs_s[(j, sub)] = psT_s
                        if j == 7:
                            p3s = tpool.tile([128, BCH], F16,
                                             tag=f"p3s{sub}",
                                             name=f"p3s{sub}")
                            nc.vector.tensor_add(
                                p3s[:], psTs_s[(6, sub)][:], psT_s[:])
                            q1s = tpool.tile([128, BCH], F16,
                                             tag=f"q1s{sub}",
                                             name=f"q1s{sub}")
                            nc.vector.tensor_add(
                                q1s[:], pairs[2][:, zsl], p3s[:])
                            prh = ps_r.tile([1, BCH], F32,
                                            tag=f"pres{sub}",
                                            name=f"pres{sub}")
                            nc.tensor.matmul(prh[:], ones_h[:],
                                             quad0[:, zsl],
                                             start=True, stop=False)
                            nc.tensor.matmul(prh[:], ones_h[:], q1s[:],
                                             start=False, stop=True)
                            b0 = r0 + sub * BCH
                            nc.scalar.activation(
                                out_sb[0:1, b0 : b0 + BCH], prh[:],
                                IDENT, bias=c_sb[0:1, 0:1])
                            nc.sync.dma_start(
                                out_2d[0:1, b0 : b0 + BCH],
                                out_sb[0:1, b0 : b0 + BCH])
                    continue
                if j % 2 == 0:
                    z16d = zpool.tile([128, 2 * SUP], F16, tag="z16d")
                    nc.scalar.activation(z16d[:, 0:SUP], p_z[:], IDENT,
                                         bias=a_sb[:, j : j + 1])
                else:
                    nc.scalar.activation(z16d[:, SUP : 2 * SUP], p_z[:],
                                         IDENT, bias=a_sb[:, j : j + 1])
                    # one double-width multiply covers psT for j-1 and j
                    psT2 = ppool.tile([128, 2 * SUP], F16, tag="psT2")
                    nc.vector.tensor_mul(psT2[:], z16d[:], oT2[j // 2])
                    pp = qpool.tile([128, SUP], F16, tag=f"pair{(j // 2) % 2}")
                    nc.vector.tensor_add(pp[:], psT2[:, 0:SUP],
                                         psT2[:, SUP : 2 * SUP])
                    pairs.append(pp)
                if sc == 0 and j <= 1:
                    # keep the HAM busy-window alive through the data
                    # ramp (p_warm's buffer is recycled at the j=2
                    # allocation, so bursts stop here)
                    for _ in range(12):
                        nc.tensor.matmul(p_warm[0:64, 0:64], warm16[:],
                                         warm16[:], start=True, stop=True)
                if j == 3:
                    quad0 = qpool.tile([128, SUP], F16, tag="quad0")
                    nc.vector.tensor_add(quad0[:], pairs[0][:], pairs[1][:])
                if j == 7:
                    q1 = qpool.tile([128, SUP], F16, tag="quad1")
                    nc.vector.tensor_add(q1[:], pairs[2][:], pairs[3][:])
                    oct_t = qpool.tile([128, SUP], F16, tag="oct")
                    nc.vector.tensor_add(oct_t[:], quad0[:], q1[:])
                    prev = (oct_t, r0)

        flush_prev()

    nc.compile()
    return nc


_CACHE: dict = {}


def _get_program(n_sup: int):
    if n_sup not in _CACHE:
        _CACHE[n_sup] = _build(n_sup)
    return _CACHE[n_sup]


def kernel(x, A, B, C, alpha, _n_sup=N_SUP, _trace=False):
    x = np.asarray(x, dtype=np.float32)
    A = np.asarray(A, dtype=np.float32)
    B = np.asarray(B, dtype=np.float32)
    C = np.asarray(C, dtype=np.float32).reshape(-1)
    alpha = np.asarray(alpha, dtype=np.float32)
    assert x.shape == (BATCH, 2 * N), x.shape

    if not np.all(alpha == 1.0):
        # Fallback (setup_inputs always produces alpha == 1): numpy eval.
        o = x[:, N:] * np.power(x[:, :N], alpha[None, :])
        return (o @ A + np.einsum("bi,ij,bj->b", o, B, o) + C[0]).astype(
            np.float32
        )

    nc = _get_program(_n_sup)

    U = np.triu(B + B.T, 1) + np.diag(np.diag(B))
    U16 = U.astype(np.float16)
    x16 = x.astype(np.float16)
    A2 = np.empty((128, N_IC), dtype=np.float32)
    for j in range(N_IC):
        A2[:, j] = A[j * 128 : (j + 1) * 128]
    C1 = np.array([[float(C[0])]], dtype=np.float32)

    rows = _n_sup * SUP
    in_maps = []
    for c in range(N_CORES):
        shard_t = np.ascontiguousarray(x16[c * SHARD : c * SHARD + rows].T)
        in_maps.append({"xt": shard_t, "u": U16, "a2": A2, "c1": C1})
    res = run_bass_kernel_spmd(
        nc, in_maps, list(range(N_CORES)), trace=_trace
    )
    if _trace:
        kernel._last_results = res
    out = np.empty(N_CORES * rows, dtype=np.float32)
    for c in range(N_CORES):
        out[c * rows : (c + 1) * rows] = res.results[c]["out"]
    if rows == SHARD:
        return out
    full = np.zeros(BATCH, dtype=np.float32)
    for c in range(N_CORES):
        full[c * SHARD : c * SHARD + rows] = out[c * rows : (c + 1) * rows]
    return full


# revision 26
# speedup vs baseline: 1.0057x; 1.0057x over previous
"""Trainium2 Bass kernel for CellSizePredictor (v10: ACT epilogue, deferred
atomic reduction, warm clock gate, triangular U loads).

reference:
    average = x[:, :n]; numbers = x[:, n:]
    o = numbers * average**alpha
    out = o @ A + einsum('bi,ij,bj->b', o, B, o) + C

Design (data-parallel over 8 cores, batch shard 8192 rows each):
  * Host pre-transposes each x shard to xT [2048, 8192] fp16; device
    streams feature-major tiles with contiguous DMAs.
  * Host folds the quadratic form into U = triu(B+B^T,1)+diag(B) so
    quad_b = sum_j o_bj (o@U)_bj; PE runs 36 of 64 [128x128x512] fp16
    matmul tiles per batch sub-chunk.
  * Epilogue off the critical engines: ACT does z16_j = Identity(p_z_j
    + A_j) from PSUM (per-partition bias); DVE does fp16 SBUF 2x ops
    only (o-mul, psT = z16*oT, pair/quad adds); ACT adds C.
  * Partition reduction: psT pairs -> quads -> one oct tile on DVE,
    then a SINGLE complete ones-row matmul group per sub-chunk.  The
    reduction matmuls + C-add + output DMA of super-chunk s are
    emitted after super-chunk s+1's j=2 group, so the PE queue never
    waits on the ACT->DVE tail chain (v5 traced a ~2us PE stall per
    boundary; split start/stop groups across the boundary wedge the
    device, so the group stays atomic).
  * Startup: super-chunk 0 uses granular per-chunk DMAs/o-muls so the
    first matmuls unlock ASAP; U arrives as chunks {0,1}, {2,3},
    {4..7}; later super-chunks use batched [128,2,SUP] pair-DMAs and
    [128, 2*SUP] o-muls (fewer Sync-queue issues).  40 dummy matmuls
    warm the PE HAM clock gate; a dummy activation pre-loads the ACT
    Identity table during the DMA ramp.
"""
import sys

for _p in ("/opt/trn_rl_repo",):
    if _p not in sys.path:
        sys.path.append(_p)

import numpy as np
from contextlib import ExitStack

import concourse.bass as bass
import concourse.tile as tile
from concourse import bacc, mybir
from concourse.bass_utils import run_bass_kernel_spmd

dt = mybir.dt
F32 = dt.float32
F16 = dt.float16

N_CORES = 8
BATCH = 65536
N = 1024
SHARD = BATCH // N_CORES          # 8192
N_IC = N // 128                   # 8 contraction chunks of 128
SUP = 1024                        # batch rows per load super-chunk
BCH = 512                         # batch rows per compute chunk (matmul N)
N_SUP = SHARD // SUP              # 8
SUB = SUP // BCH                  # 2
N_WARM = 96                       # PE warm-up dummy matmuls
IDENT = mybir.ActivationFunctionType.Identity


def _build(n_sup: int):
    nc = bacc.Bacc("TRN2", target_bir_lowering=False, debug=False)

    rows = n_sup * SUP
    x_d = nc.dram_tensor("xt", [2 * N, rows], F16, kind="ExternalInput").ap()
    u_d = nc.dram_tensor("u", [N, N], F16, kind="ExternalInput").ap()
    a_d = nc.dram_tensor("a2", [128, N_IC], F32, kind="ExternalInput").ap()
    c_d = nc.dram_tensor("c1", [1, 1], F32, kind="ExternalInput").ap()
    out_d = nc.dram_tensor("out", [rows], F32, kind="ExternalOutput").ap()
    out_2d = out_d.rearrange("(a b) -> a b", a=1)
    x_3d = x_d.rearrange("(c p) b -> p c b", p=128)   # [128, 16, rows]
    u_3d = u_d.rearrange("(c p) n -> p c n", p=128)   # [128, 8, N]

    with tile.TileContext(nc) as tc, ExitStack() as ctx:
        consts = ctx.enter_context(tc.tile_pool(name="consts", bufs=1))
        xin = ctx.enter_context(tc.tile_pool(name="xin", bufs=2))
        opool = ctx.enter_context(tc.tile_pool(name="opool", bufs=2))
        zpool = ctx.enter_context(tc.tile_pool(name="zpool", bufs=2))
        ppool = ctx.enter_context(tc.tile_pool(name="ppool", bufs=2))
        qpool = ctx.enter_context(tc.tile_pool(name="qpool", bufs=2))
        tpool = ctx.enter_context(tc.tile_pool(name="tpool", bufs=1))
        ps_z = ctx.enter_context(tc.tile_pool(name="ps_z", bufs=3, space="PSUM"))
        ps_r = ctx.enter_context(tc.tile_pool(name="ps_r", bufs=1, space="PSUM"))

        # ---- PE warm-up (HAM clock gate) + ACT table pre-load ----
        warm16 = consts.tile([128, 64], F16)
        nc.vector.memset(warm16[:], 0.0)
        p_warm = ps_z.tile([128, SUP], F32, tag="pz")
        for _ in range(N_WARM):
            nc.tensor.matmul(p_warm[0:64, 0:64], warm16[:], warm16[:],
                             start=True, stop=True)
        # ---- constants ----
        u_all = consts.tile([128, N_IC * N], F16)
        u_sb = [u_all[:, i * N : (i + 1) * N] for i in range(N_IC)]
        a_sb = consts.tile([128, N_IC], F32)
        c_sb = consts.tile([1, 1], F32)
        ones_f = consts.tile([128, 1], F32)
        nc.vector.memset(ones_f[:], 1.0)
        ones_h = consts.tile([128, 1], F16)
        nc.vector.tensor_copy(ones_h[:], ones_f[:])
        out_sb = consts.tile([1, rows], F32)
        # pre-load the ACT Identity table during the DMA ramp
        actwarm = consts.tile([1, 1], F32)
        nc.scalar.activation(actwarm[:], ones_f[0:1, 0:1], IDENT, bias=0.0)

        prev = None  # deferred tail of the previous super-chunk

        def flush_prev():
            # deferred tail of the previous super-chunk: one COMPLETE
            # single-matmul reduction group per sub (no split groups),
            # then +C on ACT and the output DMA
            nonlocal prev
            if prev is None:
                return
            oct_t, pr0 = prev
            pres = [
                ps_r.tile([1, BCH], F32, tag=f"pres{sub}", name=f"pres{sub}")
                for sub in range(SUB)
            ]
            for sub in range(SUB):
                zsl = slice(sub * BCH, (sub + 1) * BCH)
                nc.tensor.matmul(
                    pres[sub][:], ones_h[:], oct_t[:, zsl],
                    start=True, stop=True,
                )
            for sub in range(SUB):
                b0 = pr0 + sub * BCH
                nc.scalar.activation(
                    out_sb[0:1, b0 : b0 + BCH], pres[sub][:], IDENT,
                    bias=c_sb[0:1, 0:1],
                )
            nc.sync.dma_start(
                out_2d[0:1, pr0 : pr0 + SUP], out_sb[0:1, pr0 : pr0 + SUP]
            )
            prev = None

        for sc in range(n_sup):
            r0 = sc * SUP
            oT = []
            if sc == 0:
                # granular loads: chunk-at-a-time so j groups unlock ASAP
                xs = []
                os_ = []
                for i in range(N_IC):
                    if i % 2 == 0:
                        at = xin.tile([128, 2 * SUP], F16, tag=f"avg{i // 2}")
                        nt = xin.tile([128, 2 * SUP], F16, tag=f"num{i // 2}")
                        xs.append((at, nt))
                        ok = opool.tile([128, 2 * SUP], F16, tag=f"o{i // 2}")
                        os_.append(ok)
                    at, nt = xs[-1]
                    ok = os_[-1]
                    h = slice((i % 2) * SUP, (i % 2 + 1) * SUP)
                    # U chunk i, TRIANGULAR: only columns >= i*128 are
                    # ever read by the matmuls (36/64 tiles), the rest
                    # stays uninitialized — 1.125 MB instead of 2 MB
                    # on the startup-critical DMA path
                    nc.sync.dma_start(
                        u_all[:, i * N + i * 128 : (i + 1) * N],
                        u_d[i * 128 : (i + 1) * 128, i * 128 :],
                    )
                    nc.sync.dma_start(
                        at[:, h], x_d[i * 128 : (i + 1) * 128, r0 : r0 + SUP]
                    )
                    nc.sync.dma_start(
                        nt[:, h],
                        x_d[N + i * 128 : N + (i + 1) * 128, r0 : r0 + SUP],
                    )
                    if i == 0:
                        nc.sync.dma_start(a_sb[:], a_d)
                        nc.sync.dma_start(c_sb[:], c_d)
                    # per-chunk o-mul so oT_i is ready incrementally
                    nc.vector.tensor_mul(ok[:, h], at[:, h], nt[:, h])
                    oT.append(ok[:, h])
                oT2 = [t[:] for t in os_]
            else:
                # batched pair loads + double o-muls
                avg2, num2 = [], []
                for k in range(N_IC // 2):
                    at = xin.tile([128, 2 * SUP], F16, tag=f"avg{k}")
                    nc.sync.dma_start(
                        at[:].rearrange("p (c b) -> p c b", c=2),
                        x_3d[:, 2 * k : 2 * k + 2, r0 : r0 + SUP],
                    )
                    avg2.append(at)
                    nt = xin.tile([128, 2 * SUP], F16, tag=f"num{k}")
                    nc.sync.dma_start(
                        nt[:].rearrange("p (c b) -> p c b", c=2),
                        x_3d[:, N_IC + 2 * k : N_IC + 2 * k + 2,
                             r0 : r0 + SUP],
                    )
                    num2.append(nt)
                oT2 = []
                for k in range(N_IC // 2):
                    ok = opool.tile([128, 2 * SUP], F16, tag=f"o{k}")
                    nc.vector.tensor_mul(ok[:], avg2[k][:], num2[k][:])
                    oT.append(ok[:, 0:SUP])
                    oT.append(ok[:, SUP : 2 * SUP])
                    oT2.append(ok[:])

            is_last = sc == n_sup - 1
            psTs = []
            psTs_s = {}
            pairs = []
            quad0 = None
            for j in range(N_IC):
                jsl = slice(j * 128, (j + 1) * 128)
                p_z = ps_z.tile([128, SUP], F32, tag="pz")
                for sub in range(SUB):
                    zsl = slice(sub * BCH, (sub + 1) * BCH)
                    for i in range(j + 1):
                        nc.tensor.matmul(
                            p_z[:, zsl],
                            u_sb[i][:, jsl],
                            oT[i][:, zsl],
                            start=(i == 0),
                            stop=(i == j),
                        )
                if j == 5:
                    # previous super-chunk's tail rides behind our j<=4
                    # groups (Tile often runs j<=2 early, overlapped
                    # with the previous tail): flush late enough that
                    # the ACT->DVE chain is done when PE reaches it
                    flush_prev()
                if is_last and j >= 6:
                    # last super-chunk: per-sub epilogue chain so the
                    # final reduction waits on a half-width tail
                    for sub in range(SUB):
                        zsl = slice(sub * BCH, (sub + 1) * BCH)
                        z16s = tpool.tile([128, BCH], F16,
                                          tag=f"z16s{j}_{sub}",
                                          name=f"z16s{j}_{sub}")
                        nc.scalar.activation(z16s[:], p_z[:, zsl], IDENT,
                                             bias=a_sb[:, j : j + 1])
                        psT_s = tpool.tile([128, BCH], F16,
                                           tag=f"psTs{j}_{sub}",
                                           name=f"psTs{j}_{sub}")
                        nc.vector.tensor_mul(psT_s[:], z16s[:],
                                             oT[j][:, zsl])
                        psTs_s[(j, sub)] = psT_s
                        if j == 7:
                            p3s = tpool.tile([128, BCH], F16,
                                             tag=f"p3s{sub}",
                                             name=f"p3s{sub}")
                            nc.vector.tensor_add(
                                p3s[:], psTs_s[(6, sub)][:], psT_s[:])
                            q1s = tpool.tile([128, BCH], F16,
                                             tag=f"q1s{sub}",
                                             name=f"q1s{sub}")
                            nc.vector.tensor_add(
                                q1s[:], pairs[2][:, zsl], p3s[:])
                            prh = ps_r.tile([1, BCH], F32,
                                            tag=f"pres{sub}",
                                            name=f"pres{sub}")
                            nc.tensor.matmul(prh[:], ones_h[:],
                                             quad0[:, zsl],
                                             start=True, stop=False)
                            nc.tensor.matmul(prh[:], ones_h[:], q1s[:],
                                             start=False, stop=True)
                            b0 = r0 + sub * BCH
                            nc.scalar.activation(
                                out_sb[0:1, b0 : b0 + BCH], prh[:],
                                IDENT, bias=c_sb[0:1, 0:1])
                            nc.sync.dma_start(
                                out_2d[0:1, b0 : b0 + BCH],
                                out_sb[0:1, b0 : b0 + BCH])
                    continue
                if j % 2 == 0:
                    z16d = zpool.tile([128, 2 * SUP], F16, tag="z16d")
                    nc.scalar.activation(z16d[:, 0:SUP], p_z[:], IDENT,
                                         bias=a_sb[:, j : j + 1])
                else:
                    nc.scalar.activation(z16d[:, SUP : 2 * SUP], p_z[:],
                                         IDENT, bias=a_sb[:, j : j + 1])
                    # one double-width multiply covers psT for j-1 and j
                    psT2 = ppool.tile([128, 2 * SUP], F16, tag="psT2")
                    nc.vector.tensor_mul(psT2[:], z16d[:], oT2[j // 2])
                    pp = qpool.tile([128, SUP], F16, tag=f"pair{(j // 2) % 2}")
                    nc.vector.tensor_add(pp[:], psT2[:, 0:SUP],
                                         psT2[:, SUP : 2 * SUP])
                    pairs.append(pp)
                if sc == 0 and j <= 1:
                    # keep the HAM busy-window alive through the data
                    # ramp (p_warm's buffer is recycled at the j=2
                    # allocation, so bursts stop here)
                    for _ in range(12):
                        nc.tensor.matmul(p_warm[0:64, 0:64], warm16[:],
                                         warm16[:], start=True, stop=True)
                if j == 3:
                    quad0 = qpool.tile([128, SUP], F16, tag="quad0")
                    nc.vector.tensor_add(quad0[:], pairs[0][:], pairs[1][:])
                if j == 7:
                    q1 = qpool.tile([128, SUP], F16, tag="quad1")
                    nc.vector.tensor_add(q1[:], pairs[2][:], pairs[3][:])
                    oct_t = qpool.tile([128, SUP], F16, tag="oct")
                    nc.vector.tensor_add(oct_t[:], quad0[:], q1[:])
                    prev = (oct_t, r0)

        flush_prev()

    nc.compile()
    return nc


_CACHE: dict = {}


def _get_program(n_sup: int):
    if n_sup not in _CACHE:
        _CACHE[n_sup] = _build(n_sup)
    return _CACHE[n_sup]


def kernel(x, A, B, C, alpha, _n_sup=N_SUP, _trace=False):
    x = np.asarray(x, dtype=np.float32)
    A = np.asarray(A, dtype=np.float32)
    B = np.asarray(B, dtype=np.float32)
    C = np.asarray(C, dtype=np.float32).reshape(-1)
    alpha = np.asarray(alpha, dtype=np.float32)
    assert x.shape == (BATCH, 2 * N), x.shape

    if not np.all(alpha == 1.0):
        # Fallback (setup_inputs always produces alpha == 1): numpy eval.
        o = x[:, N:] * np.power(x[:, :N], alpha[None, :])
        return (o @ A + np.einsum("bi,ij,bj->b", o, B, o) + C[0]).astype(
            np.float32
        )

    nc = _get_program(_n_sup)

    U = np.triu(B + B.T, 1) + np.diag(np.diag(B))
    U16 = U.astype(np.float16)
    x16 = x.astype(np.float16)
    A2 = np.empty((128, N_IC), dtype=np.float32)
    for j in range(N_IC):
        A2[:, j] = A[j * 128 : (j + 1) * 128]
    C1 = np.array([[float(C[0])]], dtype=np.float32)

    rows = _n_sup * SUP
    in_maps = []
    for c in range(N_CORES):
        shard_t = np.ascontiguousarray(x16[c * SHARD : c * SHARD + rows].T)
        in_maps.append({"xt": shard_t, "u": U16, "a2": A2, "c1": C1})
    res = run_bass_kernel_spmd(
        nc, in_maps, list(range(N_CORES)), trace=_trace
    )
    if _trace:
        kernel._last_results = res
    out = np.empty(N_CORES * rows, dtype=np.float32)
    for c in range(N_CORES):
        out[c * rows : (c + 1) * rows] = res.results[c]["out"]
    if rows == SHARD:
        return out
    full = np.zeros(BATCH, dtype=np.float32)
    for c in range(N_CORES):
        full[c * SHARD : c * SHARD + rows] = out[c * rows : (c + 1) * rows]
    return full


# revision 28
# speedup vs baseline: 1.1300x; 1.1236x over previous
"""Trainium2 Bass kernel for CellSizePredictor (v10: ACT epilogue, deferred
atomic reduction, warm clock gate, triangular U loads).

reference:
    average = x[:, :n]; numbers = x[:, n:]
    o = numbers * average**alpha
    out = o @ A + einsum('bi,ij,bj->b', o, B, o) + C

Design (data-parallel over 8 cores, batch shard 8192 rows each):
  * Host pre-transposes each x shard to xT [2048, 8192] fp16; device
    streams feature-major tiles with contiguous DMAs.
  * Host folds the quadratic form into U = triu(B+B^T,1)+diag(B) so
    quad_b = sum_j o_bj (o@U)_bj; PE runs 36 of 64 [128x128x512] fp16
    matmul tiles per batch sub-chunk.
  * Epilogue off the critical engines: ACT does z16_j = Identity(p_z_j
    + A_j) from PSUM (per-partition bias); DVE does fp16 SBUF 2x ops
    only (o-mul, psT = z16*oT, pair/quad adds); ACT adds C.
  * Partition reduction: psT pairs -> quads -> one oct tile on DVE,
    then a SINGLE complete ones-row matmul group per sub-chunk.  The
    reduction matmuls + C-add + output DMA of super-chunk s are
    emitted after super-chunk s+1's j=2 group, so the PE queue never
    waits on the ACT->DVE tail chain (v5 traced a ~2us PE stall per
    boundary; split start/stop groups across the boundary wedge the
    device, so the group stays atomic).
  * Startup: super-chunk 0 uses granular per-chunk DMAs/o-muls so the
    first matmuls unlock ASAP; U arrives as chunks {0,1}, {2,3},
    {4..7}; later super-chunks use batched [128,2,SUP] pair-DMAs and
    [128, 2*SUP] o-muls (fewer Sync-queue issues).  40 dummy matmuls
    warm the PE HAM clock gate; a dummy activation pre-loads the ACT
    Identity table during the DMA ramp.
"""
import sys

for _p in ("/opt/trn_rl_repo",):
    if _p not in sys.path:
        sys.path.append(_p)

import numpy as np
from contextlib import ExitStack

import concourse.bass as bass
import concourse.tile as tile
from concourse import bacc, mybir
from concourse.bass_utils import run_bass_kernel_spmd

dt = mybir.dt
F32 = dt.float32
F16 = dt.float16

N_CORES = 8
BATCH = 65536
N = 1024
SHARD = BATCH // N_CORES          # 8192
N_IC = N // 128                   # 8 contraction chunks of 128
SUP = 1024                        # batch rows per load super-chunk
BCH = 512                         # batch rows per compute chunk (matmul N)
N_SUP = SHARD // SUP              # 8
SUB = SUP // BCH                  # 2
N_WARM = 96                       # PE warm-up dummy matmuls
IDENT = mybir.ActivationFunctionType.Identity


def _build(n_sup: int):
    nc = bacc.Bacc("TRN2", target_bir_lowering=False, debug=False)

    rows = n_sup * SUP
    x_d = nc.dram_tensor("xt", [2 * N, rows], F16, kind="ExternalInput").ap()
    u_d = nc.dram_tensor("u", [N, N], F16, kind="ExternalInput").ap()
    a_d = nc.dram_tensor("a2", [128, N_IC], F32, kind="ExternalInput").ap()
    c_d = nc.dram_tensor("c128", [128, 1], F32, kind="ExternalInput").ap()
    out_d = nc.dram_tensor("out", [rows], F32, kind="ExternalOutput").ap()
    out_2d = out_d.rearrange("(a b) -> a b", a=1)
    x_3d = x_d.rearrange("(c p) b -> p c b", p=128)   # [128, 16, rows]
    u_3d = u_d.rearrange("(c p) n -> p c n", p=128)   # [128, 8, N]

    with tile.TileContext(nc) as tc, ExitStack() as ctx:
        consts = ctx.enter_context(tc.tile_pool(name="consts", bufs=1))
        xin = ctx.enter_context(tc.tile_pool(name="xin", bufs=2))
        opool = ctx.enter_context(tc.tile_pool(name="opool", bufs=2))
        zpool = ctx.enter_context(tc.tile_pool(name="zpool", bufs=2))
        ppool = ctx.enter_context(tc.tile_pool(name="ppool", bufs=2))
        qpool = ctx.enter_context(tc.tile_pool(name="qpool", bufs=2))
        tpool = ctx.enter_context(tc.tile_pool(name="tpool", bufs=1))
        ps_z = ctx.enter_context(tc.tile_pool(name="ps_z", bufs=3, space="PSUM"))
        ps_r = ctx.enter_context(tc.tile_pool(name="ps_r", bufs=1, space="PSUM"))

        # ---- PE warm-up (HAM clock gate) + ACT table pre-load ----
        warm16 = consts.tile([128, 64], F16)
        nc.vector.memset(warm16[:], 0.0)
        p_warm = ps_z.tile([128, SUP], F32, tag="pz")
        for _ in range(N_WARM):
            nc.tensor.matmul(p_warm[0:64, 0:64], warm16[:], warm16[:],
                             start=True, stop=True)
        # ---- constants ----
        u_all = consts.tile([128, N_IC * N], F16)
        u_sb = [u_all[:, i * N : (i + 1) * N] for i in range(N_IC)]
        a_sb = consts.tile([128, N_IC], F32)
        c_sb = consts.tile([128, 1], F32)
        ones_f = consts.tile([128, 1], F32)
        nc.vector.memset(ones_f[:], 1.0)
        ones_h = consts.tile([128, 1], F16)
        nc.vector.tensor_copy(ones_h[:], ones_f[:])
        out_sb = consts.tile([1, rows], F32)
        # pre-load the ACT Identity table during the DMA ramp
        actwarm = consts.tile([1, 1], F32)
        nc.scalar.activation(actwarm[:], ones_f[0:1, 0:1], IDENT, bias=0.0)

        prev = None  # deferred tail of the previous super-chunk

        def flush_prev():
            # deferred tail of the previous super-chunk: one COMPLETE
            # single-matmul reduction group per sub (no split groups),
            # then +C on ACT and the output DMA
            nonlocal prev
            if prev is None:
                return
            oct_t, pr0 = prev
            pres = [
                ps_r.tile([1, BCH], F32, tag=f"pres{sub}", name=f"pres{sub}")
                for sub in range(SUB)
            ]
            for sub in range(SUB):
                zsl = slice(sub * BCH, (sub + 1) * BCH)
                nc.tensor.matmul(
                    pres[sub][:], ones_h[:], oct_t[:, zsl],
                    start=True, stop=True,
                )
                b0 = pr0 + sub * BCH
                # C already folded into oct (C/128 per partition);
                # cheap DVE copy PSUM->SBUF, then ship out
                nc.vector.tensor_copy(
                    out_sb[0:1, b0 : b0 + BCH], pres[sub][:]
                )
            nc.sync.dma_start(
                out_2d[0:1, pr0 : pr0 + SUP], out_sb[0:1, pr0 : pr0 + SUP]
            )
            prev = None

        for sc in range(n_sup):
            r0 = sc * SUP
            oT = []
            if sc == 0:
                # granular loads: chunk-at-a-time so j groups unlock ASAP
                xs = []
                os_ = []
                for i in range(N_IC):
                    if i % 2 == 0:
                        at = xin.tile([128, 2 * SUP], F16, tag=f"avg{i // 2}")
                        nt = xin.tile([128, 2 * SUP], F16, tag=f"num{i // 2}")
                        xs.append((at, nt))
                        ok = opool.tile([128, 2 * SUP], F16, tag=f"o{i // 2}")
                        os_.append(ok)
                    at, nt = xs[-1]
                    ok = os_[-1]
                    h = slice((i % 2) * SUP, (i % 2 + 1) * SUP)
                    # U chunk i, TRIANGULAR: only columns >= i*128 are
                    # ever read by the matmuls (36/64 tiles), the rest
                    # stays uninitialized — 1.125 MB instead of 2 MB
                    # on the startup-critical DMA path
                    nc.sync.dma_start(
                        u_all[:, i * N + i * 128 : (i + 1) * N],
                        u_d[i * 128 : (i + 1) * 128, i * 128 :],
                    )
                    nc.sync.dma_start(
                        at[:, h], x_d[i * 128 : (i + 1) * 128, r0 : r0 + SUP]
                    )
                    nc.sync.dma_start(
                        nt[:, h],
                        x_d[N + i * 128 : N + (i + 1) * 128, r0 : r0 + SUP],
                    )
                    if i == 0:
                        nc.sync.dma_start(a_sb[:], a_d)
                        nc.sync.dma_start(c_sb[:], c_d)
                    # per-chunk o-mul so oT_i is ready incrementally
                    nc.vector.tensor_mul(ok[:, h], at[:, h], nt[:, h])
                    oT.append(ok[:, h])
                oT2 = [t[:] for t in os_]
            else:
                # batched pair loads + double o-muls
                avg2, num2 = [], []
                for k in range(N_IC // 2):
                    at = xin.tile([128, 2 * SUP], F16, tag=f"avg{k}")
                    nc.sync.dma_start(
                        at[:].rearrange("p (c b) -> p c b", c=2),
                        x_3d[:, 2 * k : 2 * k + 2, r0 : r0 + SUP],
                    )
                    avg2.append(at)
                    nt = xin.tile([128, 2 * SUP], F16, tag=f"num{k}")
                    nc.sync.dma_start(
                        nt[:].rearrange("p (c b) -> p c b", c=2),
                        x_3d[:, N_IC + 2 * k : N_IC + 2 * k + 2,
                             r0 : r0 + SUP],
                    )
                    num2.append(nt)
                oT2 = []
                for k in range(N_IC // 2):
                    ok = opool.tile([128, 2 * SUP], F16, tag=f"o{k}")
                    nc.vector.tensor_mul(ok[:], avg2[k][:], num2[k][:])
                    oT.append(ok[:, 0:SUP])
                    oT.append(ok[:, SUP : 2 * SUP])
                    oT2.append(ok[:])

            is_last = sc == n_sup - 1
            psTs = []
            psTs_s = {}
            pairs = []
            quad0 = None
            for j in range(N_IC):
                jsl = slice(j * 128, (j + 1) * 128)
                p_z = ps_z.tile([128, SUP], F32, tag="pz")
                for sub in range(SUB):
                    zsl = slice(sub * BCH, (sub + 1) * BCH)
                    for i in range(j + 1):
                        nc.tensor.matmul(
                            p_z[:, zsl],
                            u_sb[i][:, jsl],
                            oT[i][:, zsl],
                            start=(i == 0),
                            stop=(i == j),
                        )
                if j == 5:
                    # previous super-chunk's tail rides behind our j<=4
                    # groups (Tile often runs j<=2 early, overlapped
                    # with the previous tail): flush late enough that
                    # the ACT->DVE chain is done when PE reaches it
                    flush_prev()
                if is_last and j >= 6:
                    # last super-chunk: per-sub epilogue chain so the
                    # final reduction waits on a half-width tail
                    for sub in range(SUB):
                        zsl = slice(sub * BCH, (sub + 1) * BCH)
                        z16s = tpool.tile([128, BCH], F16,
                                          tag=f"z16s{j}_{sub}",
                                          name=f"z16s{j}_{sub}")
                        nc.scalar.activation(z16s[:], p_z[:, zsl], IDENT,
                                             bias=a_sb[:, j : j + 1])
                        psT_s = tpool.tile([128, BCH], F16,
                                           tag=f"psTs{j}_{sub}",
                                           name=f"psTs{j}_{sub}")
                        nc.vector.tensor_mul(psT_s[:], z16s[:],
                                             oT[j][:, zsl])
                        psTs_s[(j, sub)] = psT_s
                        if j == 7:
                            p3s = tpool.tile([128, BCH], F16,
                                             tag=f"p3s{sub}",
                                             name=f"p3s{sub}")
                            nc.vector.tensor_add(
                                p3s[:], psTs_s[(6, sub)][:], psT_s[:])
                            q1s = tpool.tile([128, BCH], F16,
                                             tag=f"q1s{sub}",
                                             name=f"q1s{sub}")
                            nc.vector.scalar_tensor_tensor(
                                out=q1s[:], in0=pairs[2][:, zsl],
                                scalar=c_sb[:, 0:1], in1=p3s[:],
                                op0=mybir.AluOpType.add,
                                op1=mybir.AluOpType.add)
                            prh = ps_r.tile([1, BCH], F32,
                                            tag=f"pres{sub}",
                                            name=f"pres{sub}")
                            nc.tensor.matmul(prh[:], ones_h[:],
                                             quad0[:, zsl],
                                             start=True, stop=False)
                            nc.tensor.matmul(prh[:], ones_h[:], q1s[:],
                                             start=False, stop=True)
                            b0 = r0 + sub * BCH
                            nc.vector.tensor_copy(
                                out_sb[0:1, b0 : b0 + BCH], prh[:])
                            nc.sync.dma_start(
                                out_2d[0:1, b0 : b0 + BCH],
                                out_sb[0:1, b0 : b0 + BCH])
                    continue
                if j % 2 == 0:
                    z16d = zpool.tile([128, 2 * SUP], F16, tag="z16d")
                    nc.scalar.activation(z16d[:, 0:SUP], p_z[:], IDENT,
                                         bias=a_sb[:, j : j + 1])
                else:
                    nc.scalar.activation(z16d[:, SUP : 2 * SUP], p_z[:],
                                         IDENT, bias=a_sb[:, j : j + 1])
                    # one double-width multiply covers psT for j-1 and j
                    psT2 = ppool.tile([128, 2 * SUP], F16, tag="psT2")
                    nc.vector.tensor_mul(psT2[:], z16d[:], oT2[j // 2])
                    pp = qpool.tile([128, SUP], F16, tag=f"pair{(j // 2) % 2}")
                    nc.vector.tensor_add(pp[:], psT2[:, 0:SUP],
                                         psT2[:, SUP : 2 * SUP])
                    pairs.append(pp)
                if sc == 0 and j <= 1:
                    # keep the HAM busy-window alive through the data
                    # ramp (p_warm's buffer is recycled at the j=2
                    # allocation, so bursts stop here)
                    for _ in range(12):
                        nc.tensor.matmul(p_warm[0:64, 0:64], warm16[:],
                                         warm16[:], start=True, stop=True)
                if j == 3:
                    quad0 = qpool.tile([128, SUP], F16, tag="quad0")
                    nc.vector.tensor_add(quad0[:], pairs[0][:], pairs[1][:])
                if j == 7:
                    q1 = qpool.tile([128, SUP], F16, tag="quad1")
                    nc.vector.tensor_add(q1[:], pairs[2][:], pairs[3][:])
                    oct_t = qpool.tile([128, SUP], F16, tag="oct")
                    nc.vector.scalar_tensor_tensor(
                        out=oct_t[:], in0=quad0[:], scalar=c_sb[:, 0:1],
                        in1=q1[:], op0=mybir.AluOpType.add,
                        op1=mybir.AluOpType.add)
                    prev = (oct_t, r0)

        flush_prev()

    nc.compile()
    return nc


_CACHE: dict = {}


def _get_program(n_sup: int):
    if n_sup not in _CACHE:
        _CACHE[n_sup] = _build(n_sup)
    return _CACHE[n_sup]


def kernel(x, A, B, C, alpha, _n_sup=N_SUP, _trace=False):
    x = np.asarray(x, dtype=np.float32)
    A = np.asarray(A, dtype=np.float32)
    B = np.asarray(B, dtype=np.float32)
    C = np.asarray(C, dtype=np.float32).reshape(-1)
    alpha = np.asarray(alpha, dtype=np.float32)
    assert x.shape == (BATCH, 2 * N), x.shape

    if not np.all(alpha == 1.0):
        # Fallback (setup_inputs always produces alpha == 1): numpy eval.
        o = x[:, N:] * np.power(x[:, :N], alpha[None, :])
        return (o @ A + np.einsum("bi,ij,bj->b", o, B, o) + C[0]).astype(
            np.float32
        )

    nc = _get_program(_n_sup)

    U = np.triu(B + B.T, 1) + np.diag(np.diag(B))
    U16 = U.astype(np.float16)
    x16 = x.astype(np.float16)
    A2 = np.empty((128, N_IC), dtype=np.float32)
    for j in range(N_IC):
        A2[:, j] = A[j * 128 : (j + 1) * 128]
    C128 = np.full((128, 1), float(C[0]) / 128.0, dtype=np.float32)

    rows = _n_sup * SUP
    in_maps = []
    for c in range(N_CORES):
        shard_t = np.ascontiguousarray(x16[c * SHARD : c * SHARD + rows].T)
        in_maps.append({"xt": shard_t, "u": U16, "a2": A2, "c128": C128})
    res = run_bass_kernel_spmd(
        nc, in_maps, list(range(N_CORES)), trace=_trace
    )
    if _trace:
        kernel._last_results = res
    out = np.empty(N_CORES * rows, dtype=np.float32)
    for c in range(N_CORES):
        out[c * rows : (c + 1) * rows] = res.results[c]["out"]
    if rows == SHARD:
        return out
    full = np.zeros(BATCH, dtype=np.float32)
    for c in range(N_CORES):
        full[c * SHARD : c * SHARD + rows] = out[c * rows : (c + 1) * rows]
    return full


# revision 30
# speedup vs baseline: 1.1492x; 1.0170x over previous
"""Trainium2 Bass kernel for CellSizePredictor (v10: ACT epilogue, deferred
atomic reduction, warm clock gate, triangular U loads).

reference:
    average = x[:, :n]; numbers = x[:, n:]
    o = numbers * average**alpha
    out = o @ A + einsum('bi,ij,bj->b', o, B, o) + C

Design (data-parallel over 8 cores, batch shard 8192 rows each):
  * Host pre-transposes each x shard to xT [2048, 8192] fp16; device
    streams feature-major tiles with contiguous DMAs.
  * Host folds the quadratic form into U = triu(B+B^T,1)+diag(B) so
    quad_b = sum_j o_bj (o@U)_bj; PE runs 36 of 64 [128x128x512] fp16
    matmul tiles per batch sub-chunk.
  * Epilogue off the critical engines: ACT does z16_j = Identity(p_z_j
    + A_j) from PSUM (per-partition bias); DVE does fp16 SBUF 2x ops
    only (o-mul, psT = z16*oT, pair/quad adds); ACT adds C.
  * Partition reduction: psT pairs -> quads -> one oct tile on DVE,
    then a SINGLE complete ones-row matmul group per sub-chunk.  The
    reduction matmuls + C-add + output DMA of super-chunk s are
    emitted after super-chunk s+1's j=2 group, so the PE queue never
    waits on the ACT->DVE tail chain (v5 traced a ~2us PE stall per
    boundary; split start/stop groups across the boundary wedge the
    device, so the group stays atomic).
  * Startup: super-chunk 0 uses granular per-chunk DMAs/o-muls so the
    first matmuls unlock ASAP; U arrives as chunks {0,1}, {2,3},
    {4..7}; later super-chunks use batched [128,2,SUP] pair-DMAs and
    [128, 2*SUP] o-muls (fewer Sync-queue issues).  40 dummy matmuls
    warm the PE HAM clock gate; a dummy activation pre-loads the ACT
    Identity table during the DMA ramp.
"""
import sys

for _p in ("/opt/trn_rl_repo",):
    if _p not in sys.path:
        sys.path.append(_p)

import numpy as np
from contextlib import ExitStack

import concourse.bass as bass
import concourse.tile as tile
from concourse import bacc, mybir
from concourse.bass_utils import run_bass_kernel_spmd

dt = mybir.dt
F32 = dt.float32
F16 = dt.float16

N_CORES = 8
BATCH = 65536
N = 1024
SHARD = BATCH // N_CORES          # 8192
N_IC = N // 128                   # 8 contraction chunks of 128
SUP = 1024                        # batch rows per load super-chunk
BCH = 512                         # batch rows per compute chunk (matmul N)
N_SUP = SHARD // SUP              # 8
SUB = SUP // BCH                  # 2
N_WARM = 96                       # PE warm-up dummy matmuls
IDENT = mybir.ActivationFunctionType.Identity


def _build(n_sup: int):
    nc = bacc.Bacc("TRN2", target_bir_lowering=False, debug=False)

    rows = n_sup * SUP
    x_d = nc.dram_tensor("xt", [2 * N, rows], F16, kind="ExternalInput").ap()
    u_d = nc.dram_tensor("u", [N, N], F16, kind="ExternalInput").ap()
    a_d = nc.dram_tensor("a2", [128, N_IC], F32, kind="ExternalInput").ap()
    c_d = nc.dram_tensor("c128", [128, 1], F32, kind="ExternalInput").ap()
    out_d = nc.dram_tensor("out", [rows], F32, kind="ExternalOutput").ap()
    out_2d = out_d.rearrange("(a b) -> a b", a=1)
    x_3d = x_d.rearrange("(c p) b -> p c b", p=128)   # [128, 16, rows]
    u_3d = u_d.rearrange("(c p) n -> p c n", p=128)   # [128, 8, N]

    with tile.TileContext(nc) as tc, ExitStack() as ctx:
        consts = ctx.enter_context(tc.tile_pool(name="consts", bufs=1))
        xin = ctx.enter_context(tc.tile_pool(name="xin", bufs=2))
        opool = ctx.enter_context(tc.tile_pool(name="opool", bufs=2))
        zpool = ctx.enter_context(tc.tile_pool(name="zpool", bufs=2))
        ppool = ctx.enter_context(tc.tile_pool(name="ppool", bufs=2))
        qpool = ctx.enter_context(tc.tile_pool(name="qpool", bufs=2))
        tpool = ctx.enter_context(tc.tile_pool(name="tpool", bufs=1))
        ps_z = ctx.enter_context(tc.tile_pool(name="ps_z", bufs=3, space="PSUM"))
        ps_r = ctx.enter_context(tc.tile_pool(name="ps_r", bufs=1, space="PSUM"))

        # ---- PE warm-up (HAM clock gate) + ACT table pre-load ----
        warm16 = consts.tile([128, 64], F16)
        nc.vector.memset(warm16[:], 0.0)
        p_warm = ps_z.tile([128, SUP], F32, tag="pz")
        for _ in range(N_WARM):
            nc.tensor.matmul(p_warm[0:64, 0:64], warm16[:], warm16[:],
                             start=True, stop=True)
        # ---- constants ----
        u_all = consts.tile([128, N_IC * N], F16)
        u_sb = [u_all[:, i * N : (i + 1) * N] for i in range(N_IC)]
        a_sb = consts.tile([128, N_IC], F32)
        c_sb = consts.tile([128, 1], F32)
        ones_f = consts.tile([128, 1], F32)
        nc.vector.memset(ones_f[:], 1.0)
        ones_h = consts.tile([128, 1], F16)
        nc.vector.tensor_copy(ones_h[:], ones_f[:])
        out_sb = consts.tile([1, rows], F32)
        # pre-load the ACT Identity table during the DMA ramp
        actwarm = consts.tile([1, 1], F32)
        nc.scalar.activation(actwarm[:], ones_f[0:1, 0:1], IDENT, bias=0.0)

        prev = None  # deferred tail of the previous super-chunk

        def flush_prev():
            # deferred tail of the previous super-chunk: one COMPLETE
            # single-matmul reduction group per sub (no split groups),
            # then +C on ACT and the output DMA
            nonlocal prev
            if prev is None:
                return
            oct_t, pr0 = prev
            pres = [
                ps_r.tile([1, BCH], F32, tag=f"pres{sub}", name=f"pres{sub}")
                for sub in range(SUB)
            ]
            for sub in range(SUB):
                zsl = slice(sub * BCH, (sub + 1) * BCH)
                nc.tensor.matmul(
                    pres[sub][:], ones_h[:], oct_t[:, zsl],
                    start=True, stop=True,
                )
                b0 = pr0 + sub * BCH
                # C already folded into oct (C/128 per partition);
                # cheap DVE copy PSUM->SBUF, then ship out
                nc.vector.tensor_copy(
                    out_sb[0:1, b0 : b0 + BCH], pres[sub][:]
                )
            nc.sync.dma_start(
                out_2d[0:1, pr0 : pr0 + SUP], out_sb[0:1, pr0 : pr0 + SUP]
            )
            prev = None

        for sc in range(n_sup):
            r0 = sc * SUP
            oT = []
            if sc == 0:
                # granular loads: chunk-at-a-time so j groups unlock ASAP
                xs = []
                os_ = []
                for i in range(N_IC):
                    if i % 2 == 0:
                        at = xin.tile([128, 2 * SUP], F16, tag=f"avg{i // 2}")
                        nt = xin.tile([128, 2 * SUP], F16, tag=f"num{i // 2}")
                        xs.append((at, nt))
                        ok = opool.tile([128, 2 * SUP], F16, tag=f"o{i // 2}")
                        os_.append(ok)
                    at, nt = xs[-1]
                    ok = os_[-1]
                    h = slice((i % 2) * SUP, (i % 2 + 1) * SUP)
                    # U chunk i, TRIANGULAR: only columns >= i*128 are
                    # ever read by the matmuls (36/64 tiles), the rest
                    # stays uninitialized — 1.125 MB instead of 2 MB
                    # on the startup-critical DMA path
                    nc.sync.dma_start(
                        u_all[:, i * N + i * 128 : (i + 1) * N],
                        u_d[i * 128 : (i + 1) * 128, i * 128 :],
                    )
                    nc.sync.dma_start(
                        at[:, h], x_d[i * 128 : (i + 1) * 128, r0 : r0 + SUP]
                    )
                    nc.sync.dma_start(
                        nt[:, h],
                        x_d[N + i * 128 : N + (i + 1) * 128, r0 : r0 + SUP],
                    )
                    if i == 0:
                        nc.sync.dma_start(a_sb[:], a_d)
                        nc.sync.dma_start(c_sb[:], c_d)
                    # per-chunk o-mul so oT_i is ready incrementally
                    nc.vector.tensor_mul(ok[:, h], at[:, h], nt[:, h])
                    oT.append(ok[:, h])
                oT2 = [t[:] for t in os_]
            else:
                # batched pair loads + double o-muls
                avg2, num2 = [], []
                for k in range(N_IC // 2):
                    at = xin.tile([128, 2 * SUP], F16, tag=f"avg{k}")
                    nc.sync.dma_start(
                        at[:].rearrange("p (c b) -> p c b", c=2),
                        x_3d[:, 2 * k : 2 * k + 2, r0 : r0 + SUP],
                    )
                    avg2.append(at)
                    nt = xin.tile([128, 2 * SUP], F16, tag=f"num{k}")
                    nc.sync.dma_start(
                        nt[:].rearrange("p (c b) -> p c b", c=2),
                        x_3d[:, N_IC + 2 * k : N_IC + 2 * k + 2,
                             r0 : r0 + SUP],
                    )
                    num2.append(nt)
                oT2 = []
                for k in range(N_IC // 2):
                    ok = opool.tile([128, 2 * SUP], F16, tag=f"o{k}")
                    nc.vector.tensor_mul(ok[:], avg2[k][:], num2[k][:])
                    oT.append(ok[:, 0:SUP])
                    oT.append(ok[:, SUP : 2 * SUP])
                    oT2.append(ok[:])

            is_last = sc == n_sup - 1
            psTs = []
            psTs_s = {}
            pairs = []
            quad0 = None
            for j in range(N_IC):
                jsl = slice(j * 128, (j + 1) * 128)
                p_z = ps_z.tile([128, SUP], F32, tag="pz")
                for sub in range(SUB):
                    zsl = slice(sub * BCH, (sub + 1) * BCH)
                    for i in range(j + 1):
                        nc.tensor.matmul(
                            p_z[:, zsl],
                            u_sb[i][:, jsl],
                            oT[i][:, zsl],
                            start=(i == 0),
                            stop=(i == j),
                        )
                if j == 5:
                    # previous super-chunk's tail rides behind our j<=4
                    # groups (Tile often runs j<=2 early, overlapped
                    # with the previous tail): flush late enough that
                    # the ACT->DVE chain is done when PE reaches it
                    flush_prev()
                if is_last and j >= 6:
                    # last super-chunk: per-sub epilogue chain so the
                    # final reduction waits on a half-width tail
                    for sub in range(SUB):
                        zsl = slice(sub * BCH, (sub + 1) * BCH)
                        z16s = tpool.tile([128, BCH], F16,
                                          tag=f"z16s{j}_{sub}",
                                          name=f"z16s{j}_{sub}")
                        nc.scalar.activation(z16s[:], p_z[:, zsl], IDENT,
                                             bias=a_sb[:, j : j + 1])
                        psT_s = tpool.tile([128, BCH], F16,
                                           tag=f"psTs{j}_{sub}",
                                           name=f"psTs{j}_{sub}")
                        nc.vector.tensor_mul(psT_s[:], z16s[:],
                                             oT[j][:, zsl])
                        psTs_s[(j, sub)] = psT_s
                        if j == 7:
                            p3s = tpool.tile([128, BCH], F16,
                                             tag=f"p3s{sub}",
                                             name=f"p3s{sub}")
                            nc.vector.tensor_add(
                                p3s[:], psTs_s[(6, sub)][:], psT_s[:])
                            q1s = tpool.tile([128, BCH], F16,
                                             tag=f"q1s{sub}",
                                             name=f"q1s{sub}")
                            nc.vector.scalar_tensor_tensor(
                                out=q1s[:], in0=pairs[2][:, zsl],
                                scalar=c_sb[:, 0:1], in1=p3s[:],
                                op0=mybir.AluOpType.add,
                                op1=mybir.AluOpType.add)
                            prh = ps_r.tile([1, BCH], F32,
                                            tag=f"pres{sub}",
                                            name=f"pres{sub}")
                            nc.tensor.matmul(prh[:], ones_h[:],
                                             quad0[:, zsl],
                                             start=True, stop=False)
                            nc.tensor.matmul(prh[:], ones_h[:], q1s[:],
                                             start=False, stop=True)
                            b0 = r0 + sub * BCH
                            nc.vector.tensor_copy(
                                out_sb[0:1, b0 : b0 + BCH], prh[:])
                            nc.sync.dma_start(
                                out_2d[0:1, b0 : b0 + BCH],
                                out_sb[0:1, b0 : b0 + BCH])
                    continue
                if j % 2 == 0:
                    z16d = zpool.tile([128, 2 * SUP], F16, tag="z16d")
                    nc.scalar.activation(z16d[:, 0:SUP], p_z[:], IDENT,
                                         bias=a_sb[:, j : j + 1])
                else:
                    nc.scalar.activation(z16d[:, SUP : 2 * SUP], p_z[:],
                                         IDENT, bias=a_sb[:, j : j + 1])
                    # one double-width multiply covers psT for j-1 and j
                    psT2 = ppool.tile([128, 2 * SUP], F16, tag="psT2")
                    nc.vector.tensor_mul(psT2[:], z16d[:], oT2[j // 2])
                    pp = qpool.tile([128, SUP], F16, tag=f"pair{(j // 2) % 2}")
                    nc.vector.tensor_add(pp[:], psT2[:, 0:SUP],
                                         psT2[:, SUP : 2 * SUP])
                    pairs.append(pp)
                if sc == 0 and j <= 1:
                    # keep the HAM busy-window alive through the data
                    # ramp (p_warm's buffer is recycled at the j=2
                    # allocation, so bursts stop here)
                    for _ in range(12):
                        nc.tensor.matmul(p_warm[0:64, 0:64], warm16[:],
                                         warm16[:], start=True, stop=True)
                if j == 3:
                    quad0 = qpool.tile([128, SUP], F16, tag="quad0")
                    nc.vector.tensor_add(quad0[:], pairs[0][:], pairs[1][:])
                if j == 7:
                    q1 = qpool.tile([128, SUP], F16, tag="quad1")
                    nc.vector.tensor_add(q1[:], pairs[2][:], pairs[3][:])
                    oct_t = qpool.tile([128, SUP], F16, tag="oct")
                    nc.vector.scalar_tensor_tensor(
                        out=oct_t[:], in0=quad0[:], scalar=c_sb[:, 0:1],
                        in1=q1[:], op0=mybir.AluOpType.add,
                        op1=mybir.AluOpType.add)
                    prev = (oct_t, r0)

        flush_prev()

    nc.compile()
    return nc


_CACHE: dict = {}


def _get_program(n_sup: int):
    if n_sup not in _CACHE:
        _CACHE[n_sup] = _build(n_sup)
    return _CACHE[n_sup]


def kernel(x, A, B, C, alpha, _n_sup=N_SUP, _trace=False):
    x = np.asarray(x, dtype=np.float32)
    A = np.asarray(A, dtype=np.float32)
    B = np.asarray(B, dtype=np.float32)
    C = np.asarray(C, dtype=np.float32).reshape(-1)
    alpha = np.asarray(alpha, dtype=np.float32)
    assert x.shape == (BATCH, 2 * N), x.shape

    if not np.all(alpha == 1.0):
        # Fallback (setup_inputs always produces alpha == 1): numpy eval.
        o = x[:, N:] * np.power(x[:, :N], alpha[None, :])
        return (o @ A + np.einsum("bi,ij,bj->b", o, B, o) + C[0]).astype(
            np.float32
        )

    nc = _get_program(_n_sup)

    U = np.triu(B + B.T, 1) + np.diag(np.diag(B))
    U16 = U.astype(np.float16)
    x16 = x.astype(np.float16)
    A2 = np.empty((128, N_IC), dtype=np.float32)
    for j in range(N_IC):
        A2[:, j] = A[j * 128 : (j + 1) * 128]
    C128 = np.full((128, 1), float(C[0]) / 128.0, dtype=np.float32)

    rows = _n_sup * SUP
    in_maps = []
    for c in range(N_CORES):
        shard_t = np.ascontiguousarray(x16[c * SHARD : c * SHARD + rows].T)
        in_maps.append({"xt": shard_t, "u": U16, "a2": A2, "c128": C128})
    res = run_bass_kernel_spmd(
        nc, in_maps, list(range(N_CORES)), trace=_trace
    )
    if _trace:
        kernel._last_results = res
    out = np.empty(N_CORES * rows, dtype=np.float32)
    for c in range(N_CORES):
        out[c * rows : (c + 1) * rows] = res.results[c]["out"]
    if rows == SHARD:
        return out
    full = np.zeros(BATCH, dtype=np.float32)
    for c in range(N_CORES):
        full[c * SHARD : c * SHARD + rows] = out[c * rows : (c + 1) * rows]
    return full
